# revision 9
# baseline (speedup 1.0000x reference)
"""Trainium2 Bass kernel for Bahdanau-style attention scoring (sparse_attention).

Math (per reference):
    u1 = W[:, :H].T @ v ; u2 = W[:, H:].T @ v ; c = b @ v
    sh[b, n] = hidden[n, b, :] @ u1
    se[b, t] = encoder_outputs[t, b, :] @ u2
    out[b, n, t] = softmax_t(tanh(sh[b, n] + se[b, t] + c))

Sharding: data-parallel over batch B=64 across 8 cores (8 batch rows per
core); W/b/v replicated. No collectives.

Per-core plan (B_loc=8, N=256, T=1024, H=256), all device-side:
  - setup: small PE matmuls compute u1/u2 rows, broadcast them across
    partitions, and compute c (broadcast to 128 partitions).
  - per b: DMA enc_b (128p x [8,256]) and hid_b (128p x [2,256]) natural
    (1KB contiguous rows).
  - VectorE tensor_tensor_reduce: se columns (128,8) and sh+c columns
    (128,2) per b.
  - PE transpose se columns -> (8,128) rows; ScalarE copy to SBUF.
  - PE K=1 broadcast matmuls: pre[n_part, t] = 1 (x) se_b -> PSUM
    (128,1024), shared by both n-tiles of this b.
  - ScalarE: tanh(pre + bias=sh_col) -> e ; exp(e) with accum_out -> row
    sums.  VectorE: reciprocal + per-partition scale.  DMA out rows
    (4KB contiguous per row).
"""

import os
import sys

import numpy as np

for _p in ("/opt/trn_rl_repo", "/root/.axon_site/_ro/trn_rl_repo"):
    if os.path.isdir(_p) and _p not in sys.path:
        sys.path.insert(0, _p)

from contextlib import ExitStack

import concourse.bass as bass
import concourse.tile as tile
from concourse import bacc, mybir
from concourse.bass_utils import run_bass_kernel_spmd

H = 256
N_LEN = 256
T_LEN = 1024
BATCH = 64
NCORES = 8
B_LOC = BATCH // NCORES  # 8
P = 128
FP32 = mybir.dt.float32
AF = mybir.ActivationFunctionType
ALU = mybir.AluOpType


def build_program():
    nc = bacc.Bacc(
        "TRN2",
        target_bir_lowering=False,
        debug=False,
        enable_asserts=True,
        num_devices=NCORES,
    )

    hid_ap = nc.dram_tensor("hidden", [N_LEN, B_LOC, H], FP32, kind="ExternalInput").ap()
    enc_ap = nc.dram_tensor(
        "encoder_outputs", [T_LEN, B_LOC, H], FP32, kind="ExternalInput"
    ).ap()
    w_ap = nc.dram_tensor("W", [H, 2 * H], FP32, kind="ExternalInput").ap()
    b_ap = nc.dram_tensor("b", [H], FP32, kind="ExternalInput").ap()
    v_ap = nc.dram_tensor("v", [H], FP32, kind="ExternalInput").ap()
    out_ap = nc.dram_tensor(
        "out", [B_LOC, N_LEN, T_LEN], FP32, kind="ExternalOutput"
    ).ap()

    # DRAM views: partition index innermost of the row dims.
    hid_r = hid_ap.rearrange("(j p) b h -> p b j h", p=P)  # (128, 8, 2, 256)
    enc_r = enc_ap.rearrange("(k p) b h -> p b k h", p=P)  # (128, 8, 8, 256)
    w_r = w_ap.rearrange("(k p) j -> p k j", p=P)  # (128, 2, 512)
    v_r = v_ap.rearrange("(k p) -> p k", p=P)  # (128, 2)
    b_r = b_ap.rearrange("(k p) -> p k", p=P)  # (128, 2)
    out_r = out_ap.rearrange("b (j p) t -> b j p t", p=P)  # (8, 2, 128, 1024)

    with tile.TileContext(nc) as tc, ExitStack() as ctx:
        singles = ctx.enter_context(tc.tile_pool(name="singles", bufs=1))
        ps_setup = ctx.enter_context(tc.tile_pool(name="ps_setup", bufs=1, space="PSUM"))
        ps_set = ctx.enter_context(tc.tile_pool(name="ps_set", bufs=2, space="PSUM"))
        ps_pre = ctx.enter_context(tc.tile_pool(name="ps_pre", bufs=2, space="PSUM"))
        enc_pool = ctx.enter_context(tc.tile_pool(name="enc", bufs=2))
        hid_pool = ctx.enter_context(tc.tile_pool(name="hid", bufs=2))
        small = ctx.enter_context(tc.tile_pool(name="small", bufs=3))
        scratch_pool = ctx.enter_context(tc.tile_pool(name="scratch", bufs=4))
        et_pool = ctx.enter_context(tc.tile_pool(name="et", bufs=2))
        xt_pool = ctx.enter_context(tc.tile_pool(name="xt", bufs=2))
        ot_pool = ctx.enter_context(tc.tile_pool(name="ot", bufs=3))
        dram_pool = ctx.enter_context(tc.tile_pool(name="dscr", bufs=2, space="DRAM"))

        # ---- constants / weights ----
        w_sb = singles.tile([P, 2, 2 * H], FP32)
        nc.sync.dma_start(w_sb[:], w_r)
        v_sb = singles.tile([P, 2], FP32)
        nc.sync.dma_start(v_sb[:], v_r)
        b_sb = singles.tile([P, 2], FP32)
        nc.sync.dma_start(b_sb[:], b_r)

        ones_sb = singles.tile([1, P], FP32)
        nc.vector.memset(ones_sb[:], 1.0)

        # ---- u1/u2 rows: (1, 256) = v.T @ W[:, half] ----
        u_rows_sb = singles.tile([1, 2, H], FP32)
        for half in range(2):
            u_ps = ps_setup.tile([1, H], FP32, tag="u_ps")
            for k in range(2):
                nc.tensor.matmul(
                    out=u_ps[:],
                    lhsT=v_sb[:, k : k + 1],
                    rhs=w_sb[:, k, half * H : (half + 1) * H],
                    start=(k == 0),
                    stop=(k == 1),
                )
            nc.scalar.copy(u_rows_sb[:, half, :], u_ps[:])

        # ---- c = b @ v, broadcast to 128 partitions ----
        c_ps = ps_setup.tile([1, 1], FP32, tag="u_ps")
        for k in range(2):
            nc.tensor.matmul(
                out=c_ps[:],
                lhsT=b_sb[:, k : k + 1],
                rhs=v_sb[:, k : k + 1],
                start=(k == 0),
                stop=(k == 1),
            )
        c_sb = singles.tile([1, 1], FP32)
        nc.scalar.copy(c_sb[:], c_ps[:])
        c128_ps = ps_setup.tile([P, 1], FP32, tag="u_ps")
        nc.tensor.matmul(
            out=c128_ps[:], lhsT=ones_sb[:], rhs=c_sb[:], start=True, stop=True
        )
        c128_sb = singles.tile([P, 1], FP32)
        nc.scalar.copy(c128_sb[:], c128_ps[:])

        # ---- u broadcast across partitions: (128, 256) each ----
        u_bcast = []
        for half in range(2):
            ub_ps = ps_setup.tile([P, H], FP32, tag="u_ps")
            nc.tensor.matmul(
                out=ub_ps[:],
                lhsT=ones_sb[:],
                rhs=u_rows_sb[:, half, :],
                start=True,
                stop=True,
            )
            ub_sb = singles.tile([P, H], FP32, tag=f"u_bcast{half}")
            nc.scalar.copy(ub_sb[:], ub_ps[:])
            u_bcast.append(ub_sb)
        u1_bcast, u2_bcast = u_bcast

        # ---- main loop over local batch ----
        for b in range(B_LOC):
            enc_sb = enc_pool.tile([P, 8, H], FP32)
            nc.sync.dma_start(enc_sb[:], enc_r[:, b])
            hid_sb = hid_pool.tile([P, 2, H], FP32)
            nc.sync.dma_start(hid_sb[:], hid_r[:, b])

            # se columns: (128, 8); sh+c columns: (128, 2)
            se_cols = small.tile([P, 8], FP32)
            for k in range(8):
                scr = scratch_pool.tile([P, H], FP32, tag="scr")
                nc.vector.scalar_tensor_tensor(
                    out=scr[:],
                    in0=enc_sb[:, k, :],
                    scalar=1.0,
                    in1=u2_bcast[:],
                    op0=ALU.bypass,
                    op1=ALU.mult,
                    accum_out=se_cols[:, k : k + 1],
                )
            shc_cols = small.tile([P, 2], FP32)
            for j in range(2):
                scr = scratch_pool.tile([P, H], FP32, tag="scr")
                nc.vector.scalar_tensor_tensor(
                    out=scr[:],
                    in0=hid_sb[:, j, :],
                    scalar=1.0,
                    in1=u1_bcast[:],
                    op0=ALU.bypass,
                    op1=ALU.mult,
                    accum_out=shc_cols[:, j : j + 1],
                )
            # fold in the bias constant c = b @ v
            nc.vector.tensor_scalar_add(shc_cols[:], shc_cols[:], c128_sb[:, 0:1])

            # reshape se columns (128, 8) -> one row (1, 1024); element (p, k)
            # lands at t = k*128 + p.  Round-trip via a small DRAM scratch
            # (SBUF->SBUF cross-partition reshape exceeds DMA AP dims).
            se_scr = dram_pool.tile([T_LEN], FP32)
            nc.sync.dma_start(se_scr[:].rearrange("(k p) -> p k", p=P), se_cols[:])
            se_row = small.tile([1, T_LEN], FP32)
            nc.sync.dma_start(se_row[:], se_scr[:])

            # pre[n_part, t] = se_b[t] broadcast across partitions (PSUM)
            pre_ps = ps_pre.tile([P, T_LEN], FP32)
            for half in range(2):
                nc.tensor.matmul(
                    out=pre_ps[:, half * 512 : (half + 1) * 512],
                    lhsT=ones_sb[:],
                    rhs=se_row[0:1, half * 512 : (half + 1) * 512],
                    start=True,
                    stop=True,
                )

            sums = small.tile([P, 2], FP32)
            rsums = small.tile([P, 2], FP32)
            for j in range(2):
                e_t = et_pool.tile([P, T_LEN], FP32)
                nc.scalar.activation(
                    out=e_t[:],
                    in_=pre_ps[:],
                    func=AF.Tanh,
                    bias=shc_cols[:, j : j + 1],
                    scale=1.0,
                )
                x_t = xt_pool.tile([P, T_LEN], FP32)
                nc.scalar.activation(
                    out=x_t[:],
                    in_=e_t[:],
                    func=AF.Exp,
                    accum_out=sums[:, j : j + 1],
                )
                nc.vector.reciprocal(rsums[:, j : j + 1], sums[:, j : j + 1])
                o_t = ot_pool.tile([P, T_LEN], FP32)
                nc.vector.tensor_scalar_mul(o_t[:], x_t[:], rsums[:, j : j + 1])
                nc.sync.dma_start(out_r[b, j], o_t[:])

    nc.compile()
    return nc


_CACHE = {}


def get_program():
    if "nc" not in _CACHE:
        _CACHE["nc"] = build_program()
    return _CACHE["nc"]


def make_in_maps(hidden, encoder_outputs, W, b, v):
    in_maps = []
    for i in range(NCORES):
        sl = slice(i * B_LOC, (i + 1) * B_LOC)
        in_maps.append(
            {
                "hidden": np.ascontiguousarray(hidden[:, sl, :], dtype=np.float32),
                "encoder_outputs": np.ascontiguousarray(
                    encoder_outputs[:, sl, :], dtype=np.float32
                ),
                "W": np.ascontiguousarray(W, dtype=np.float32),
                "b": np.ascontiguousarray(b, dtype=np.float32),
                "v": np.ascontiguousarray(v, dtype=np.float32),
            }
        )
    return in_maps


def kernel(hidden, encoder_outputs, W, b, v, _trace=False, _trace_kwargs=None):
    nc = get_program()
    in_maps = make_in_maps(hidden, encoder_outputs, W, b, v)
    res = run_bass_kernel_spmd(
        nc,
        in_maps,
        core_ids=list(range(NCORES)),
        trace=_trace,
        **(_trace_kwargs or {}),
    )
    out = np.concatenate([res.results[i]["out"] for i in range(NCORES)], axis=0)
    if _trace:
        return out, res
    return out
